# revision 10
# baseline (speedup 1.0000x reference)
"""Trainium2 Bass kernel for nn_InteractionBlock (gnn_message_passing).

Algebraic collapse: per angle alpha with (s, t) = (src, tgt):
    sm[alpha] = (msg[s] @ Ws + bs) * d[t]
    out[alpha] = sum_b a[t, b] * (Wb[:, b, :] @ sm[alpha])
    agg[t] = sum_{alpha: tgt=t} out[alpha]
Everything except msg[s] depends only on t, so with
    S[t] = sum_{alpha: tgt=t} msg[s(alpha)]   and  c[t] = |{alpha: tgt=t}|
    agg[t] = sum_b a[t,b] * (Wb[:,b,:] @ ((S[t] @ Ws + c[t]*bs) * d[t]))
S is a sparse-matrix product (host, scipy CSR); the dense E-sized pipeline
runs on 8 cores, edges sharded contiguously, bf16 feature-major with f32
PSUM accumulation.

Wall-time notes (the metric includes everything in kernel()): the axon
host<->device link has a large per-array cost and ~30-45 MB/s throughput,
so all per-core inputs are packed into TWO bf16 tensors (DATA activations,
WTS weights), and a background thread touches all 8 devices immediately so
device attach/cleanup overlaps host preprocessing and compilation.
"""

import sys
import threading
import time as _time

import numpy as np
import ml_dtypes

sys.path.insert(0, "/opt/trn_rl_repo")

BF16 = ml_dtypes.bfloat16

E = 100000
A = 600000
NR = 6
NS = 7
H = 128
BD = 8
M = 128
P = 8            # cores
ES = E // P      # 12500 edges per core
NT = 512         # dense-phase column tile
NSP = 12800      # padded edges per core (25 * 512)
NTILES = NSP // NT  # 25

# DATA row layout (partition bases chosen so matmul lhsT/rhs bases match)
R_MSG = 0        # rows   0-127: msg^T
R_S = 128        # rows 128-255: S^T (segment sums)
R_A = 256        # rows 256-263: a^T (angle proj), base 0 of small tile
R_XD = 288       # rows 288-293: x_dist^T, base 32 of small tile
R_CNT = 320      # row 320: counts, base 64 of small tile
DROWS = 321

# WTS column layout (bf16 [128, WCOLS])
C_WS = 0
C_WT = 128
C_RB1 = 256
C_RB2 = 384
C_WSK = 512
C_RA11 = 640
C_RA12 = 768
C_RA21 = 896
C_RA22 = 1024
C_WBT = 1152          # [128, 1024]
C_SEL = 2176          # rows 0-7, [8, 1024]
C_WD = 3200           # rows 32-37, [6, 128]
C_BS = 3328           # row 64, [1, 128]
C_BT = 3456           # row 0, [1, 128]
C_BIAS = 3584         # rows 0-127, 7 cols: rb_b1, rb_b2, bskip, ra1_b1,
                      # ra1_b2, ra2_b1, ra2_b2
WCOLS = 3600


def _start_device_warm():
    """Touch all 8 devices on a daemon thread so attach/cleanup cost
    overlaps host-side preprocessing and compilation."""
    def _warm():
        try:
            import jax
            for d in jax.devices():
                jax.device_put(np.zeros(8, np.float32), d).block_until_ready()
        except Exception:
            pass
    th = threading.Thread(target=_warm, daemon=True)
    th.start()
    return th


def _preprocess(x_dist, x_angle, msg, angle_index, Wa):
    """Host prep: segment-sum S (sparse matmul), a-projection, bf16 packing
    into one [DROWS, NSP] tensor per core."""
    src = np.ascontiguousarray(angle_index[0]).astype(np.int32, copy=False)
    tgt = np.ascontiguousarray(angle_index[1]).astype(np.int64, copy=False)

    order = np.argsort(tgt, kind="stable")
    cnt = np.bincount(tgt, minlength=E).astype(np.int64)
    indptr = np.zeros(E + 1, np.int64)
    np.cumsum(cnt, out=indptr[1:])
    from scipy.sparse import csr_matrix
    C = csr_matrix((np.ones(A, np.float32), src[order], indptr), shape=(E, E))
    S = C @ msg                                   # [E, M] f32 segment sums

    a = x_angle.reshape(E, NS * NR) @ Wa          # [E, BD] f32

    data = np.zeros((P, DROWS, NSP), BF16)

    def put(row, x, k):
        data[:, row:row + k, :ES] = x.reshape(P, ES, k).transpose(0, 2, 1)

    put(R_MSG, msg, M)
    put(R_S, S, M)
    put(R_A, a, BD)
    put(R_XD, x_dist, NR)
    put(R_CNT, cnt.astype(np.float32)[:, None], 1)
    return data


def _pack_weights(Wd, Ws, bs, Wt, bt, Wb, rb_w1, rb_b1, rb_w2, rb_b2,
                  Wskip, bskip, ra1_w1, ra1_b1, ra1_w2, ra1_b2,
                  ra2_w1, ra2_b1, ra2_w2, ra2_b2):
    w = np.zeros((128, WCOLS), BF16)
    for c, mat in ((C_WS, Ws), (C_WT, Wt), (C_RB1, rb_w1), (C_RB2, rb_w2),
                   (C_WSK, Wskip), (C_RA11, ra1_w1), (C_RA12, ra1_w2),
                   (C_RA21, ra2_w1), (C_RA22, ra2_w2)):
        w[:, c:c + 128] = mat
    for b in range(BD):
        w[:, C_WBT + b * 128:C_WBT + (b + 1) * 128] = Wb[:, b, :].T
        w[b, C_SEL + b * 128:C_SEL + (b + 1) * 128] = 1.0
    w[32:38, C_WD:C_WD + 128] = Wd
    w[64, C_BS:C_BS + 128] = bs
    w[0, C_BT:C_BT + 128] = bt
    for j, vec in enumerate((rb_b1, rb_b2, bskip, ra1_b1, ra1_b2,
                             ra2_b1, ra2_b2)):
        w[:, C_BIAS + j] = vec
    return w


def _build(nc, tc, aps):
    from contextlib import ExitStack

    from concourse import mybir

    bf16 = mybir.dt.bfloat16
    f32 = mybir.dt.float32
    Silu = mybir.ActivationFunctionType.Silu
    mult = mybir.AluOpType.mult

    with ExitStack() as ctx:
        wpool = ctx.enter_context(tc.tile_pool(name="w", bufs=1))

        wtile = wpool.tile([128, WCOLS], bf16, tag="wts")
        nc.sync.dma_start(wtile[:], aps["WTS"][:])
        bias = wpool.tile([128, 7], f32, tag="bias")
        nc.scalar.copy(bias[:], wtile[:, C_BIAS:C_BIAS + 7])
        ones_row = wpool.tile([1, NT], bf16, tag="ones")
        nc.gpsimd.memset(ones_row[:], 1.0)

        def W(c, k=128):          # [128, k] weight slice
            return wtile[:, c:c + k]

        dense = ctx.enter_context(tc.tile_pool(name="dn", bufs=3))
        pacc = ctx.enter_context(tc.tile_pool(name="pacc", bufs=2, space="PSUM"))
        psc = ctx.enter_context(tc.tile_pool(name="psc", bufs=4, space="PSUM"))

        def mm(out, lhsT, rhs, start=True, stop=True):
            nc.tensor.matmul(out[:], lhsT=lhsT[:], rhs=rhs[:], start=start,
                             stop=stop, skip_group_check=True)

        from concourse.bass import ds

        with tc.For_i(0, NSP, NT) as iv:
            sl = ds(iv, NT)

            msgT_t = dense.tile([M, NT], bf16, tag="msgT")
            nc.sync.dma_start(msgT_t[:], aps["DATA"][R_MSG:R_MSG + M, sl])
            ST_t = dense.tile([M, NT], bf16, tag="ST")
            nc.sync.dma_start(ST_t[:], aps["DATA"][R_S:R_S + M, sl])
            rest_t = dense.tile([65, NT], bf16, tag="rest")
            nc.sync.dma_start(rest_t[:], aps["DATA"][R_A:R_A + 65, sl])
            aT_t = rest_t[0:BD, :]
            xdT_t = rest_t[32:32 + NR, :]
            cnt_t = rest_t[64:65, :]

            # d = x_dist @ Wd
            ps_d = psc.tile([H, NT], f32, tag="ps")
            mm(ps_d, wtile[32:38, C_WD:C_WD + 128], xdT_t)
            d_sb = dense.tile([H, NT], bf16, tag="d")
            nc.scalar.copy(d_sb[:], ps_d[:])

            # u = (S@Ws + c*bs) * d
            ps_u = psc.tile([H, NT], f32, tag="ps")
            mm(ps_u, W(C_WS), ST_t, start=True, stop=False)
            mm(ps_u, wtile[64:65, C_BS:C_BS + 128], cnt_t, start=False,
               stop=True)
            u_sb = dense.tile([H, NT], bf16, tag="u")
            nc.vector.tensor_tensor(out=u_sb[:], in0=ps_u[:], in1=d_sb[:],
                                    op=mult)

            # x0 = agg + msg@Wt + bt
            ps_x0 = pacc.tile([H, NT], f32, tag="pacc")
            mm(ps_x0, W(C_WT), msgT_t, start=True, stop=False)
            mm(ps_x0, wtile[0:1, C_BT:C_BT + 128], ones_row, start=False,
               stop=False)
            for b in range(BD):
                # broadcast a[:, b] across partitions via one-hot selector
                ps_a = psc.tile([H, NT], f32, tag="ps")
                mm(ps_a, wtile[0:BD, C_SEL + b * 128:C_SEL + (b + 1) * 128],
                   aT_t)
                z_sb = dense.tile([H, NT], bf16, tag="z")
                nc.vector.tensor_tensor(out=z_sb[:], in0=u_sb[:], in1=ps_a[:],
                                        op=mult)
                mm(ps_x0, W(C_WBT + b * 128), z_sb, start=False,
                   stop=(b == BD - 1))
            x0_sb = dense.tile([H, NT], bf16, tag="x0")
            nc.scalar.copy(x0_sb[:], ps_x0[:])

            # residual block (H)
            ps_h = psc.tile([H, NT], f32, tag="ps")
            mm(ps_h, W(C_RB1), x0_sb)
            h1_sb = dense.tile([H, NT], bf16, tag="h1")
            nc.scalar.activation(h1_sb[:], ps_h[:], Silu, bias=bias[:, 0:1])
            ps_h2 = psc.tile([H, NT], f32, tag="ps")
            mm(ps_h2, W(C_RB2), h1_sb)
            h2_sb = dense.tile([H, NT], bf16, tag="h2")
            nc.scalar.activation(h2_sb[:], ps_h2[:], Silu, bias=bias[:, 1:2])

            # skip: y = silu((x0+h2)@Wskip + bskip) + msg
            ps_y = pacc.tile([H, NT], f32, tag="pacc")
            mm(ps_y, W(C_WSK), x0_sb, start=True, stop=False)
            mm(ps_y, W(C_WSK), h2_sb, start=False, stop=True)
            ys_sb = dense.tile([M, NT], bf16, tag="ys")
            nc.scalar.activation(ys_sb[:], ps_y[:], Silu, bias=bias[:, 2:3])
            y_sb = dense.tile([M, NT], bf16, tag="y")
            nc.vector.tensor_add(out=y_sb[:], in0=ys_sb[:], in1=msgT_t[:])

            # residual after 1
            ps_h = psc.tile([M, NT], f32, tag="ps")
            mm(ps_h, W(C_RA11), y_sb)
            h1p = dense.tile([M, NT], bf16, tag="h1")
            nc.scalar.activation(h1p[:], ps_h[:], Silu, bias=bias[:, 3:4])
            ps_h2 = psc.tile([M, NT], f32, tag="ps")
            mm(ps_h2, W(C_RA12), h1p)
            h2p = dense.tile([M, NT], bf16, tag="h2")
            nc.scalar.activation(h2p[:], ps_h2[:], Silu, bias=bias[:, 4:5])
            x2_sb = dense.tile([M, NT], bf16, tag="x2")
            nc.vector.tensor_add(out=x2_sb[:], in0=y_sb[:], in1=h2p[:])

            # residual after 2
            ps_h = psc.tile([M, NT], f32, tag="ps")
            mm(ps_h, W(C_RA21), x2_sb)
            h1q = dense.tile([M, NT], bf16, tag="h1")
            nc.scalar.activation(h1q[:], ps_h[:], Silu, bias=bias[:, 5:6])
            ps_h2 = psc.tile([M, NT], f32, tag="ps")
            mm(ps_h2, W(C_RA22), h1q)
            h2q = dense.tile([M, NT], bf16, tag="h2")
            nc.scalar.activation(h2q[:], ps_h2[:], Silu, bias=bias[:, 6:7])
            o_sb = dense.tile([M, NT], bf16, tag="o")
            nc.vector.tensor_add(out=o_sb[:], in0=x2_sb[:], in1=h2q[:])

            nc.sync.dma_start(aps["outT"][:, sl], o_sb[:])


def kernel(**inputs):
    _t0 = _time.time()

    def _tick(label):
        print(f"[kernel-timing] {label}: {_time.time() - _t0:.2f}s",
              file=sys.stderr, flush=True)

    warm = _start_device_warm()

    inputs = {k: np.asarray(v) for k, v in inputs.items()}
    x_dist = np.asarray(inputs["x_dist"], np.float32)
    x_angle = np.asarray(inputs["x_angle"], np.float32)
    msg = np.asarray(inputs["msg"], np.float32)
    angle_index = inputs["angle_index"]

    data = _preprocess(x_dist, x_angle, msg, angle_index,
                       np.asarray(inputs["Wa"], np.float32))
    wts = _pack_weights(**{k: np.asarray(inputs[k], np.float32) for k in (
        "Wd", "Ws", "bs", "Wt", "bt", "Wb",
        "rb_w1", "rb_b1", "rb_w2", "rb_b2", "Wskip", "bskip",
        "ra1_w1", "ra1_b1", "ra1_w2", "ra1_b2",
        "ra2_w1", "ra2_b1", "ra2_w2", "ra2_b2")})
    in_maps = [{"DATA": data[p], "WTS": wts} for p in range(P)]
    _tick("preprocess")

    import concourse.tile as tile
    from concourse import bacc, mybir
    from concourse import bass_utils

    nc = bacc.Bacc("TRN2", target_bir_lowering=False, debug=False,
                   enable_asserts=False, num_devices=P)
    aps = {
        "DATA": nc.dram_tensor("DATA", (DROWS, NSP), mybir.dt.bfloat16,
                               kind="ExternalInput").ap(),
        "WTS": nc.dram_tensor("WTS", (128, WCOLS), mybir.dt.bfloat16,
                              kind="ExternalInput").ap(),
        "outT": nc.dram_tensor("outT", (M, NSP), mybir.dt.bfloat16,
                               kind="ExternalOutput").ap(),
    }

    with tile.TileContext(nc) as tc:
        _build(nc, tc, aps)
    _tick("graph-build")
    nc.compile()
    _tick("bass-compile")

    warm.join(timeout=300)
    _tick("warm-join")

    res = bass_utils.run_bass_kernel_spmd(nc, in_maps, core_ids=list(range(P)))
    kernel.last_results = res
    _tick("run-spmd")

    out = np.empty((E, M), np.float32)
    for p in range(P):
        outT = res.results[p]["outT"]  # [M, NSP] bf16
        out[p * ES:(p + 1) * ES] = outT[:, :ES].T.astype(np.float32)
    _tick("reassemble")
    return out


# revision 20
# speedup vs baseline: 7.5564x; 7.5564x over previous
"""Trainium2 Bass kernel for nn_InteractionBlock (gnn_message_passing).

Algebraic collapse: per angle alpha with (s, t) = (src, tgt):
    sm[alpha] = (msg[s] @ Ws + bs) * d[t]
    out[alpha] = sum_b a[t, b] * (Wb[:, b, :] @ sm[alpha])
    agg[t] = sum_{alpha: tgt=t} out[alpha]
Everything except msg[s] depends only on t, so with
    S[t] = sum_{alpha: tgt=t} msg[s(alpha)]   and  c[t] = |{alpha: tgt=t}|
    agg[t] = sum_b a[t,b] * (Wb[:,b,:] @ ((S[t] @ Ws + c[t]*bs) * d[t]))
S is a sparse-matrix product (host, scipy CSR); the dense E-sized pipeline
runs on 8 cores, edges sharded contiguously, bf16 feature-major with f32
PSUM accumulation.

Wall-time notes (the metric includes everything in kernel()): the axon
host<->device link has a large per-array cost and ~30-45 MB/s throughput,
so all per-core inputs are packed into TWO bf16 tensors (DATA activations,
WTS weights), and a background thread touches all 8 devices immediately so
device attach/cleanup overlaps host preprocessing and compilation.
"""

import sys
import threading
import time as _time

import numpy as np
import ml_dtypes

sys.path.insert(0, "/opt/trn_rl_repo")

BF16 = ml_dtypes.bfloat16

E = 100000
A = 600000
NR = 6
NS = 7
H = 128
BD = 8
M = 128
P = 8            # cores
ES = E // P      # 12500 edges per core
NT = 512         # dense-phase column tile
NSP = 12800      # padded edges per core (25 * 512)
NTILES = NSP // NT  # 25

# DATA row layout
R_MSG = 0        # rows   0-127: msg^T
R_S = 128        # rows 128-255: S^T (segment sums)
R_A = 256        # rows 256-263: a^T (angle projection)
R_XD = 264       # rows 264-269: x_dist^T
R_CNT = 270      # row 270: counts
DROWS = 271

# WTS column layout (bf16 [128, WCOLS])
C_WS = 0
C_WT = 128
C_RB1 = 256
C_RB2 = 384
C_WSK = 512
C_RA11 = 640
C_RA12 = 768
C_RA21 = 896
C_RA22 = 1024
C_WBT = 1152          # [128, 1024]
C_SEL = 2176          # rows 0-7, [8, 1024]
C_WD = 3200           # rows 0-5, [6, 128]
C_BS = 3328           # row 0, [1, 128]
C_BT = 3456           # row 0, [1, 128]
C_BIAS = 3584         # rows 0-127, 7 cols: rb_b1, rb_b2, bskip, ra1_b1,
                      # ra1_b2, ra2_b1, ra2_b2
WCOLS = 3600


def _start_device_warm():
    """Touch all 8 devices on a daemon thread so attach/cleanup cost
    overlaps host-side preprocessing and compilation."""
    def _warm():
        try:
            import jax
            for d in jax.devices():
                jax.device_put(np.zeros(8, np.float32), d).block_until_ready()
        except Exception:
            pass
    th = threading.Thread(target=_warm, daemon=True)
    th.start()
    return th


def _start_import_prefetch():
    """Load the heavy concourse/jax modules on a daemon thread so import
    time overlaps the numpy/scipy preprocessing."""
    def _imp():
        try:
            import concourse.tile  # noqa: F401
            from concourse import bacc, bass_utils, bass2jax  # noqa: F401
            import scipy.sparse  # noqa: F401
        except Exception:
            pass
    th = threading.Thread(target=_imp, daemon=True)
    th.start()
    return th


def _preprocess(x_dist, x_angle, msg, angle_index, Wa):
    """Host prep: segment-sum S (sparse matmul), a-projection, bf16 packing
    into one [DROWS, NSP] tensor per core."""
    src = np.ascontiguousarray(angle_index[0]).astype(np.int32, copy=False)
    tgt = np.ascontiguousarray(angle_index[1]).astype(np.int64, copy=False)

    order = np.argsort(tgt, kind="stable")
    cnt = np.bincount(tgt, minlength=E).astype(np.int64)
    indptr = np.zeros(E + 1, np.int64)
    np.cumsum(cnt, out=indptr[1:])
    from scipy.sparse import csr_matrix
    C = csr_matrix((np.ones(src.shape[0], np.float32), src[order], indptr),
                   shape=(E, E))
    S = C @ msg                                   # [E, M] f32 segment sums

    a = x_angle.reshape(E, NS * NR) @ Wa          # [E, BD] f32

    data = np.zeros((P, DROWS, NSP), BF16)

    def put(row, x, k):
        data[:, row:row + k, :ES] = x.reshape(P, ES, k).transpose(0, 2, 1)

    put(R_MSG, msg, M)
    put(R_S, S, M)
    put(R_A, a, BD)
    put(R_XD, x_dist, NR)
    put(R_CNT, cnt.astype(np.float32)[:, None], 1)
    return data


def _pack_weights(Wd, Ws, bs, Wt, bt, Wb, rb_w1, rb_b1, rb_w2, rb_b2,
                  Wskip, bskip, ra1_w1, ra1_b1, ra1_w2, ra1_b2,
                  ra2_w1, ra2_b1, ra2_w2, ra2_b2):
    w = np.zeros((128, WCOLS), BF16)
    for c, mat in ((C_WS, Ws), (C_WT, Wt), (C_RB1, rb_w1), (C_RB2, rb_w2),
                   (C_WSK, Wskip), (C_RA11, ra1_w1), (C_RA12, ra1_w2),
                   (C_RA21, ra2_w1), (C_RA22, ra2_w2)):
        w[:, c:c + 128] = mat
    for b in range(BD):
        w[:, C_WBT + b * 128:C_WBT + (b + 1) * 128] = Wb[:, b, :].T
        w[b, C_SEL + b * 128:C_SEL + (b + 1) * 128] = 1.0
    w[0:6, C_WD:C_WD + 128] = Wd
    w[0, C_BS:C_BS + 128] = bs
    w[0, C_BT:C_BT + 128] = bt
    for j, vec in enumerate((rb_b1, rb_b2, bskip, ra1_b1, ra1_b2,
                             ra2_b1, ra2_b2)):
        w[:, C_BIAS + j] = vec
    return w


def _build(nc, tc, aps):
    from contextlib import ExitStack

    from concourse import mybir

    bf16 = mybir.dt.bfloat16
    f32 = mybir.dt.float32
    Silu = mybir.ActivationFunctionType.Silu
    mult = mybir.AluOpType.mult

    with ExitStack() as ctx:
        wpool = ctx.enter_context(tc.tile_pool(name="w", bufs=1))

        wtile = wpool.tile([128, WCOLS], bf16, tag="wts")
        nc.sync.dma_start(wtile[:], aps["WTS"][:])
        bias = wpool.tile([128, 7], f32, tag="bias")
        nc.scalar.copy(bias[:], wtile[:, C_BIAS:C_BIAS + 7])
        ones_row = wpool.tile([1, NT], bf16, tag="ones")
        nc.gpsimd.memset(ones_row[:], 1.0)

        def W(c, k=128):          # [128, k] weight slice
            return wtile[:, c:c + k]

        dense = ctx.enter_context(tc.tile_pool(name="dn", bufs=3))
        pacc = ctx.enter_context(tc.tile_pool(name="pacc", bufs=2, space="PSUM"))
        psc = ctx.enter_context(tc.tile_pool(name="psc", bufs=4, space="PSUM"))

        def mm(out, lhsT, rhs, start=True, stop=True):
            nc.tensor.matmul(out[:], lhsT=lhsT[:], rhs=rhs[:], start=start,
                             stop=stop, skip_group_check=True)

        from concourse.bass import ds

        with tc.For_i(0, NSP, NT) as iv:
            sl = ds(iv, NT)

            msgT_t = dense.tile([M, NT], bf16, tag="msgT")
            nc.sync.dma_start(msgT_t[:], aps["DATA"][R_MSG:R_MSG + M, sl])
            ST_t = dense.tile([M, NT], bf16, tag="ST")
            nc.sync.dma_start(ST_t[:], aps["DATA"][R_S:R_S + M, sl])
            aT_t = dense.tile([BD, NT], bf16, tag="aT")
            nc.sync.dma_start(aT_t[:], aps["DATA"][R_A:R_A + BD, sl])
            xdT_t = dense.tile([NR, NT], bf16, tag="xdT")
            nc.sync.dma_start(xdT_t[:], aps["DATA"][R_XD:R_XD + NR, sl])
            cnt_t = dense.tile([1, NT], bf16, tag="cnt")
            nc.sync.dma_start(cnt_t[:], aps["DATA"][R_CNT:R_CNT + 1, sl])

            # d = x_dist @ Wd
            ps_d = psc.tile([H, NT], f32, tag="ps")
            mm(ps_d, wtile[0:6, C_WD:C_WD + 128], xdT_t)
            d_sb = dense.tile([H, NT], bf16, tag="d")
            nc.scalar.copy(d_sb[:], ps_d[:])

            # u = (S@Ws + c*bs) * d
            ps_u = psc.tile([H, NT], f32, tag="ps")
            mm(ps_u, W(C_WS), ST_t, start=True, stop=False)
            mm(ps_u, wtile[0:1, C_BS:C_BS + 128], cnt_t, start=False,
               stop=True)
            u_sb = dense.tile([H, NT], bf16, tag="u")
            nc.vector.tensor_tensor(out=u_sb[:], in0=ps_u[:], in1=d_sb[:],
                                    op=mult)

            # x0 = agg + msg@Wt + bt
            ps_x0 = pacc.tile([H, NT], f32, tag="pacc")
            mm(ps_x0, W(C_WT), msgT_t, start=True, stop=False)
            mm(ps_x0, wtile[0:1, C_BT:C_BT + 128], ones_row, start=False,
               stop=False)
            for b in range(BD):
                # broadcast a[:, b] across partitions via one-hot selector
                ps_a = psc.tile([H, NT], f32, tag="ps")
                mm(ps_a, wtile[0:BD, C_SEL + b * 128:C_SEL + (b + 1) * 128],
                   aT_t)
                z_sb = dense.tile([H, NT], bf16, tag="z")
                nc.vector.tensor_tensor(out=z_sb[:], in0=u_sb[:], in1=ps_a[:],
                                        op=mult)
                mm(ps_x0, W(C_WBT + b * 128), z_sb, start=False,
                   stop=(b == BD - 1))
            x0_sb = dense.tile([H, NT], bf16, tag="x0")
            nc.scalar.copy(x0_sb[:], ps_x0[:])

            # residual block (H)
            ps_h = psc.tile([H, NT], f32, tag="ps")
            mm(ps_h, W(C_RB1), x0_sb)
            h1_sb = dense.tile([H, NT], bf16, tag="h1")
            nc.scalar.activation(h1_sb[:], ps_h[:], Silu, bias=bias[:, 0:1])
            ps_h2 = psc.tile([H, NT], f32, tag="ps")
            mm(ps_h2, W(C_RB2), h1_sb)
            h2_sb = dense.tile([H, NT], bf16, tag="h2")
            nc.scalar.activation(h2_sb[:], ps_h2[:], Silu, bias=bias[:, 1:2])

            # skip: y = silu((x0+h2)@Wskip + bskip) + msg
            ps_y = pacc.tile([H, NT], f32, tag="pacc")
            mm(ps_y, W(C_WSK), x0_sb, start=True, stop=False)
            mm(ps_y, W(C_WSK), h2_sb, start=False, stop=True)
            ys_sb = dense.tile([M, NT], bf16, tag="ys")
            nc.scalar.activation(ys_sb[:], ps_y[:], Silu, bias=bias[:, 2:3])
            y_sb = dense.tile([M, NT], bf16, tag="y")
            nc.vector.tensor_add(out=y_sb[:], in0=ys_sb[:], in1=msgT_t[:])

            # residual after 1
            ps_h = psc.tile([M, NT], f32, tag="ps")
            mm(ps_h, W(C_RA11), y_sb)
            h1p = dense.tile([M, NT], bf16, tag="h1")
            nc.scalar.activation(h1p[:], ps_h[:], Silu, bias=bias[:, 3:4])
            ps_h2 = psc.tile([M, NT], f32, tag="ps")
            mm(ps_h2, W(C_RA12), h1p)
            h2p = dense.tile([M, NT], bf16, tag="h2")
            nc.scalar.activation(h2p[:], ps_h2[:], Silu, bias=bias[:, 4:5])
            x2_sb = dense.tile([M, NT], bf16, tag="x2")
            nc.vector.tensor_add(out=x2_sb[:], in0=y_sb[:], in1=h2p[:])

            # residual after 2
            ps_h = psc.tile([M, NT], f32, tag="ps")
            mm(ps_h, W(C_RA21), x2_sb)
            h1q = dense.tile([M, NT], bf16, tag="h1")
            nc.scalar.activation(h1q[:], ps_h[:], Silu, bias=bias[:, 5:6])
            ps_h2 = psc.tile([M, NT], f32, tag="ps")
            mm(ps_h2, W(C_RA22), h1q)
            h2q = dense.tile([M, NT], bf16, tag="h2")
            nc.scalar.activation(h2q[:], ps_h2[:], Silu, bias=bias[:, 6:7])
            o_sb = dense.tile([M, NT], bf16, tag="o")
            nc.vector.tensor_add(out=o_sb[:], in0=x2_sb[:], in1=h2q[:])

            nc.sync.dma_start(aps["outT"][:, sl], o_sb[:])


def kernel(**inputs):
    _t0 = _time.time()

    def _tick(label):
        print(f"[kernel-timing] {label}: {_time.time() - _t0:.2f}s",
              file=sys.stderr, flush=True)

    warm = _start_device_warm()
    imp = _start_import_prefetch()

    inputs = {k: np.asarray(v) for k, v in inputs.items()}
    x_dist = np.asarray(inputs["x_dist"], np.float32)
    x_angle = np.asarray(inputs["x_angle"], np.float32)
    msg = np.asarray(inputs["msg"], np.float32)
    angle_index = inputs["angle_index"]

    data = _preprocess(x_dist, x_angle, msg, angle_index,
                       np.asarray(inputs["Wa"], np.float32))
    wts = _pack_weights(**{k: np.asarray(inputs[k], np.float32) for k in (
        "Wd", "Ws", "bs", "Wt", "bt", "Wb",
        "rb_w1", "rb_b1", "rb_w2", "rb_b2", "Wskip", "bskip",
        "ra1_w1", "ra1_b1", "ra1_w2", "ra1_b2",
        "ra2_w1", "ra2_b1", "ra2_w2", "ra2_b2")})
    in_maps = [{"DATA": data[p], "WTS": wts} for p in range(P)]
    _tick("preprocess")

    imp.join(timeout=300)
    import concourse.tile as tile
    from concourse import bacc, mybir
    from concourse import bass_utils

    nc = bacc.Bacc("TRN2", target_bir_lowering=False, debug=False,
                   enable_asserts=False, num_devices=P)
    aps = {
        "DATA": nc.dram_tensor("DATA", (DROWS, NSP), mybir.dt.bfloat16,
                               kind="ExternalInput").ap(),
        "WTS": nc.dram_tensor("WTS", (128, WCOLS), mybir.dt.bfloat16,
                              kind="ExternalInput").ap(),
        "outT": nc.dram_tensor("outT", (M, NSP), mybir.dt.bfloat16,
                               kind="ExternalOutput").ap(),
    }

    with tile.TileContext(nc) as tc:
        _build(nc, tc, aps)
    _tick("graph-build")
    nc.compile()
    _tick("bass-compile")

    warm.join(timeout=300)
    _tick("warm-join")

    res = bass_utils.run_bass_kernel_spmd(nc, in_maps, core_ids=list(range(P)))
    kernel.last_results = res
    _tick("run-spmd")

    out = np.empty((E, M), np.float32)
    for p in range(P):
        outT = res.results[p]["outT"]  # [M, NSP] bf16
        out[p * ES:(p + 1) * ES] = outT[:, :ES].T.astype(np.float32)
    _tick("reassemble")

    # Release device-side state (executables, buffers) so the next process
    # attaches to clean devices instead of paying a ~30s lazy-cleanup stall.
    try:
        import gc
        import jax
        kernel.last_results = res
        del res
        gc.collect()
        jax.clear_caches()
        jax.device_put(np.zeros(8, np.float32)).block_until_ready()
    except Exception:
        pass
    _tick("cleanup")
    return out
